# revision 43
# baseline (speedup 1.0000x reference)
"""Trainium2 Bass kernel for nn_DecoderLayer (self-attn + cross-attn + FFN).

Sharding: 8 cores = (batch b in 0..3) x (query-half in 0..1). Each core
computes 512 query tokens of one batch element end-to-end; K/V projections
over the full source sequence are duplicated across the two halves of a
batch element, so no collectives are needed.

Per-core layout strategy:
  - activations kept feature-major (x^T: [D, tokens]) so every linear layer
    uses the weight matrix as stored (lhsT = W[k_chunk, out_chunk]).
  - attention scores computed transposed ([s, t]); softmax runs without
    max-subtraction (scores are O(1); masked entries are -1e20 -> exp = 0).
  - V computed token-major with an appended ones column so the ctx matmul
    also produces the softmax denominator; normalization is a per-partition
    tensor_scalar multiply on eviction.
  - LayerNorm runs token-major (bn_stats/bn_aggr along the free dim); the
    LN output is PE-transposed back to feature-major for the next stage.

Self-contained: hardcodes all shapes; no sibling imports.
"""

import numpy as np
import ml_dtypes
from contextlib import ExitStack

import concourse.bass as bass
import concourse.tile as tile
from concourse import bacc, mybir
from concourse.bass_utils import run_bass_kernel_spmd
from concourse.masks import make_identity

P = 128
LN_EPS = 1e-5

F32 = mybir.dt.float32

AF = mybir.ActivationFunctionType
ALU = mybir.AluOpType


def build_decoder_nc(D=1024, S=1024, TP=512, H=16, FF=4096, mm_dt=F32,
                     dram_mm_dt=None):
    """Build the per-core SPMD program.

    D: model dim; S: source seq len (= full T); TP: query tokens per core;
    H: heads (dh fixed 64); FF: ffn inner dim. mm_dt: dtype used for matmul
    operands (float32 or float32r). dram_mm_dt: dtype used to DECLARE the
    DRAM tensors that only feed matmuls (float32r trick); defaults to mm_dt.
    """
    dh = 64
    assert D % P == 0 and S % P == 0 and TP % P == 0 and FF % P == 0
    assert H * dh == D
    KC = D // P          # contraction chunks over D
    SB = S // P          # source blocks
    TB = TP // P         # query-token blocks
    NQ = TP              # free size of q/scores matmuls (<= 512)
    assert NQ <= 512
    VW = min(512, D)     # v-proj free width
    VH = D // VW
    OW = min(512, D)     # out-proj free width
    ODH = D // OW
    FFC = FF // P
    HPV = VW // dh       # heads per v-proj chunk

    if dram_mm_dt is None:
        dram_mm_dt = mm_dt

    nc = bacc.Bacc("TRN2", target_bir_lowering=False, debug=False)

    def din(name, shape, dt=F32):
        return nc.dram_tensor(name, shape, dt, kind="ExternalInput").ap()

    xqT = din("xqT", [D, TP], dram_mm_dt)     # queries slice, feature-major
    xq = din("xq", [TP, D])                   # queries slice, token-major
    xfT = din("xfT", [D, S], dram_mm_dt)      # full x[b], feature-major
    encT = din("encT", [D, S], dram_mm_dt)    # enc_out[b], feature-major
    m1T = din("m1T", [S, TP], mybir.dt.bfloat16)   # additive tgt mask [s,t]
    m2T = din("m2T", [S, TP], mybir.dt.bfloat16)   # additive src mask [s,t]
    wq1 = din("wq1", [D, D], dram_mm_dt)      # pre-scaled by dh**-0.5
    wkv1 = din("wkv1", [D, 2 * D], dram_mm_dt)
    wo1 = din("wo1", [D, D], dram_mm_dt)
    wq2 = din("wq2", [D, D], dram_mm_dt)
    wkv2 = din("wkv2", [D, 2 * D], dram_mm_dt)
    wo2 = din("wo2", [D, D], dram_mm_dt)
    w_in = din("w_in", [D, FF], dram_mm_dt)
    w_out = din("w_out", [FF, D], dram_mm_dt)
    out = nc.dram_tensor("out", [TP, D], F32, kind="ExternalOutput").ap()

    with tile.TileContext(nc) as tc:
        with ExitStack() as ctx:
            # ---- persistent pools ----
            consts = ctx.enter_context(tc.tile_pool(name="consts", bufs=1))
            p_res = ctx.enter_context(tc.tile_pool(name="p_res", bufs=3))
            p_wl = ctx.enter_context(tc.tile_pool(name="p_wl", bufs=3))
            p_wr = ctx.enter_context(tc.tile_pool(name="p_wr", bufs=8))
            p_stat = ctx.enter_context(tc.tile_pool(name="p_stat", bufs=10))
            p_msk = ctx.enter_context(tc.tile_pool(name="p_msk", bufs=2))
            pp_big = ctx.enter_context(
                tc.tile_pool(name="pp_big", bufs=4, space="PSUM"))
            pp_ctx = ctx.enter_context(
                tc.tile_pool(name="pp_ctx", bufs=2, space="PSUM"))
            pp_tr = ctx.enter_context(
                tc.tile_pool(name="pp_tr", bufs=2, space="PSUM"))

            ident = consts.tile([P, P], F32)
            make_identity(nc, ident)
            eps_t = consts.tile([P, 1], F32)
            nc.vector.memset(eps_t, LN_EPS)

            def layernorm(res, xout):
                """token-major LN: res/xout are [P, TB, D] tiles."""
                nsub = max(1, D // 512)
                w = D // nsub
                for tb in range(TB):
                    st = p_stat.tile([P, nsub, 6], F32, tag="lnst")
                    for g in range(nsub):
                        nc.vector.bn_stats(st[:, g, :],
                                           res[:, tb, g * w:(g + 1) * w])
                    mv = p_stat.tile([P, 2], F32, tag="lnmv")
                    nc.vector.bn_aggr(mv, st)
                    std = p_stat.tile([P, 1], F32, tag="lnstd")
                    nc.scalar.activation(std, mv[:, 1:2], AF.Sqrt, bias=eps_t)
                    rstd = p_stat.tile([P, 1], F32, tag="lnrstd")
                    nc.vector.reciprocal(rstd, std)
                    nc.vector.tensor_scalar(
                        out=xout[:, tb, :], in0=res[:, tb, :],
                        scalar1=mv[:, 0:1], scalar2=rstd,
                        op0=ALU.subtract, op1=ALU.mult)

            def transpose_to_fm(src, dstT):
                """src [P, TB, D] token-major -> dstT [P, KC, TP] feature-major."""
                for tb in range(TB):
                    for fc in range(KC):
                        ps = pp_tr.tile([P, P], F32)
                        nc.tensor.transpose(
                            ps, src[:, tb, fc * P:(fc + 1) * P], ident)
                        nc.vector.tensor_copy(
                            dstT[:, fc, tb * P:(tb + 1) * P], ps)

            def wl_col(w_d, c0, rows=D):
                """one DMA: [rows, P] weight column block as lhsT chunks
                [P, kc, P]."""
                wt = p_wl.tile([P, rows // P, P], mm_dt, tag="wl", name="wl")
                nc.sync.dma_start(
                    wt, w_d[:, c0:c0 + P].rearrange("(kc p) m -> p kc m", p=P))
                return wt

            def attn_stage(sctx, kvT_dram, q_src_T, wq_d, wkv_d, wo_d,
                           m_dram, res_in):
                """One attention block. Returns (x_out, x_outT-producer fn).

                kvT_dram: [D, S] feature-major dram AP for k/v source.
                q_src_T: either ("dram", AP [D, TP]) or ("tile", sbuf tile
                         [P, KC, TP]) for the feature-major query source.
                res_in: token-major [P, TB, D] residual source tile, or
                        ("dram", xq AP) for stage 1.
                """
                kvr = kvT_dram.rearrange("(c p) s -> p c s", p=P)

                # -- projections: kT, v (+ones), qT --
                kT = sctx.enter_context(
                    tc.tile_pool(name="kT", bufs=1)).tile([P, KC, S], mm_dt)
                vt = sctx.enter_context(
                    tc.tile_pool(name="vt", bufs=1)).tile(
                        [P, SB, H, dh + 1], F32)
                qT = sctx.enter_context(
                    tc.tile_pool(name="qT", bufs=1)).tile([P, KC, NQ], mm_dt)
                ones_c = consts.tile([P, H, 1], F32, tag="ones_c")
                nc.vector.memset(ones_c, 1.0)
                for sb in range(SB):
                    # rounding producer for the f32r ones column
                    nc.vector.tensor_copy(vt[:, sb, :, dh:dh + 1], ones_c)

                with tc.tile_pool(name="kv_src", bufs=1) as p_src, \
                        tc.tile_pool(name="q_src", bufs=3) as p_qsrc:
                    kvsrc = p_src.tile([P, KC, S], mm_dt)
                    for kc in range(KC):
                        for sh2 in range(2):
                            nc.sync.dma_start(
                                kvsrc[:, kc, sh2 * S // 2:(sh2 + 1) * S // 2],
                                kvr[:, kc, sh2 * S // 2:(sh2 + 1) * S // 2])

                    # kT: feature-major k = wk.T @ x^T
                    SH = S // 512 if S >= 512 else 1
                    SW = S // SH
                    for ofg in range(0, KC, 2):
                        ofs = range(ofg, min(ofg + 2, KC))
                        pss = {}
                        for of in ofs:
                            for sh in range(SH):
                                pss[(of, sh)] = pp_big.tile(
                                    [P, SW], F32, tag="ps", name="ps")
                        wts = {of: wl_col(wkv_d, of * P) for of in ofs}
                        for kc in range(KC):
                            for of in ofs:
                                for sh in range(SH):
                                    nc.tensor.matmul(
                                        pss[(of, sh)], wts[of][:, kc, :],
                                        kvsrc[:, kc, sh * SW:(sh + 1) * SW],
                                        start=(kc == 0), stop=(kc == KC - 1))
                        for of in ofs:
                            for sh in range(SH):
                                nc.scalar.copy(
                                    kT[:, of, sh * SW:(sh + 1) * SW],
                                    pss[(of, sh)])

                    # v token-major: v = x @ wv, heads interleaved, +1s col
                    SBG = 4 if SB % 4 == 0 else SB
                    for vh in range(VH):
                        for sbg in range(0, SB, SBG):
                            sbs = range(sbg, min(sbg + SBG, SB))
                            pss = {sb: pp_big.tile([P, VW], F32, tag="ps", name="ps")
                                   for sb in sbs}
                            for kc in range(KC):
                                wr = p_wr.tile([P, VW], mm_dt)
                                nc.sync.dma_start(
                                    wr, wkv_d[kc * P:(kc + 1) * P,
                                              D + vh * VW:D + (vh + 1) * VW])
                                for sb in sbs:
                                    nc.tensor.matmul(
                                        pss[sb],
                                        kvsrc[:, kc, sb * P:(sb + 1) * P],
                                        wr, start=(kc == 0),
                                        stop=(kc == KC - 1))
                            for sb in sbs:
                                nc.scalar.copy(
                                    vt[:, sb, vh * HPV:(vh + 1) * HPV, 0:dh],
                                    pss[sb].rearrange("p (h d) -> p h d",
                                                      d=dh))

                    # qT feature-major
                    if q_src_T[0] == "dram":
                        qsr = q_src_T[1].rearrange("(c p) t -> p c t", p=P)
                        qsrc = p_qsrc.tile([P, KC, NQ], mm_dt, tag="qsrc",
                                           bufs=1)
                        for kc in range(KC):
                            nc.sync.dma_start(qsrc[:, kc, :], qsr[:, kc, :])
                        qt_src = qsrc
                    else:
                        qt_src = q_src_T[1]
                    for ofg in range(0, KC, 2):
                        ofs = range(ofg, min(ofg + 2, KC))
                        pss = {of: pp_big.tile([P, NQ], F32, tag="ps",
                                               name="ps") for of in ofs}
                        wts = {of: wl_col(wq_d, of * P) for of in ofs}
                        for kc in range(KC):
                            for of in ofs:
                                nc.tensor.matmul(
                                    pss[of], wts[of][:, kc, :],
                                    qt_src[:, kc, :],
                                    start=(kc == 0), stop=(kc == KC - 1))
                        for of in ofs:
                            nc.scalar.copy(qT[:, of, :], pss[of])

                # -- per-head attention --
                ctxt = p_res.tile([P, TB, D], F32, tag="res")
                with tc.tile_pool(name="mT", bufs=1) as p_mT, \
                        tc.tile_pool(name="expp", bufs=20) as p_exp:
                    mT = p_mT.tile([P, SB, NQ], mybir.dt.bfloat16)
                    nc.sync.dma_start(
                        mT, m_dram.rearrange("(sb p) t -> p sb t", p=P))
                    # heads paired: consecutive K=64 score matmuls land on
                    # disjoint PE row groups (base_partition 0 / 64) and run
                    # concurrently.
                    for hp in range(0, H, 2):
                        pair = list(range(hp, min(hp + 2, H)))
                        ets = {}
                        for sb in range(SB):
                            for h in pair:
                                kc_h, ko = divmod(h * dh, P)
                                ps = pp_big.tile([P, NQ], F32, tag="ps",
                                                 name="ps")
                                nc.tensor.matmul(
                                    ps,
                                    kT[ko:ko + dh, kc_h, sb * P:(sb + 1) * P],
                                    qT[ko:ko + dh, kc_h, :],
                                    start=True, stop=True)
                                nc.vector.tensor_add(ps, ps, mT[:, sb, :])
                                et = p_exp.tile([P, NQ], F32, name="et")
                                nc.scalar.activation(et, ps, AF.Exp)
                                ets[(h, sb)] = et
                        for h in pair:
                            for tb in range(TB):
                                psc = pp_ctx.tile([P, dh + 1], F32, name="psc")
                                for sb in range(SB):
                                    nc.tensor.matmul(
                                        psc,
                                        ets[(h, sb)][:, tb * P:(tb + 1) * P],
                                        vt[:, sb, h, :],
                                        start=(sb == 0), stop=(sb == SB - 1))
                                rec = p_stat.tile([P, 1], F32, tag="rec",
                                                  name="rec")
                                nc.vector.reciprocal(rec, psc[:, dh:dh + 1])
                                nc.vector.tensor_scalar_mul(
                                    ctxt[:, tb, h * dh:(h + 1) * dh],
                                    in0=psc[:, 0:dh], scalar1=rec)

                # -- transpose ctx to feature-major --
                res = p_res.tile([P, TB, D], F32, tag="res")
                with tc.tile_pool(name="ctxT", bufs=1) as p_ctxT:
                    ctxT = p_ctxT.tile([P, KC, TP], mm_dt)
                    for tb in range(TB):
                        for fc in range(KC):
                            ps = pp_tr.tile([P, P], F32)
                            nc.tensor.transpose(
                                ps, ctxt[:, tb, fc * P:(fc + 1) * P], ident)
                            nc.vector.tensor_copy(
                                ctxT[:, fc, tb * P:(tb + 1) * P], ps)

                    # -- out-projection + residual --
                    if res_in[0] == "dram":
                        ri = p_res.tile([P, TB, D], F32, tag="res")
                        nc.sync.dma_start(
                            ri, res_in[1].rearrange("(tb p) d -> p tb d", p=P))
                        rsrc = ri
                    else:
                        rsrc = res_in[1]
                    for oh in range(ODH):
                        pss = {tb: pp_big.tile([P, OW], F32, tag="ps",
                                               name="ps") for tb in range(TB)}
                        for fc in range(KC):
                            wr = p_wr.tile([P, OW], mm_dt)
                            nc.sync.dma_start(
                                wr, wo_d[fc * P:(fc + 1) * P,
                                         oh * OW:(oh + 1) * OW])
                            for tb in range(TB):
                                nc.tensor.matmul(
                                    pss[tb], ctxT[:, fc, tb * P:(tb + 1) * P],
                                    wr, start=(fc == 0), stop=(fc == KC - 1))
                        for tb in range(TB):
                            nc.vector.tensor_add(
                                res[:, tb, oh * OW:(oh + 1) * OW], pss[tb],
                                rsrc[:, tb, oh * OW:(oh + 1) * OW])

                xo = p_res.tile([P, TB, D], F32, tag="res")
                layernorm(res, xo)
                xoT = p_res.tile([P, KC, TP], mm_dt, tag="res")
                transpose_to_fm(xo, xoT)
                return xo, xoT

            # ---------------- stage 1: self-attention ----------------
            with ExitStack() as s1:
                x1, x1T = attn_stage(s1, xfT, ("dram", xqT), wq1, wkv1, wo1,
                                     m1T, ("dram", xq))

            # ---------------- stage 2: cross-attention ----------------
            with ExitStack() as s2:
                x2, x2T = attn_stage(s2, encT, ("tile", x1T), wq2, wkv2, wo2,
                                     m2T, ("tile", x1))

            # ---------------- stage 3: FFN ----------------
            with tc.tile_pool(name="hT", bufs=1) as p_hT:
                hT = p_hT.tile([P, FFC, NQ], mm_dt)
                for ffc in range(FFC):
                    ps = pp_big.tile([P, NQ], F32, tag="ps", name="ps")
                    wt = wl_col(w_in, ffc * P)
                    for kc in range(KC):
                        nc.tensor.matmul(ps, wt[:, kc, :], x2T[:, kc, :],
                                         start=(kc == 0), stop=(kc == KC - 1))
                    nc.scalar.activation(hT[:, ffc, :], ps, AF.Relu)

                res3 = p_res.tile([P, TB, D], F32, tag="res")
                FFG = 8 if FFC % 8 == 0 else FFC
                for oh in range(ODH):
                    pss = {tb: pp_big.tile([P, OW], F32, tag="ps", name="ps")
                           for tb in range(TB)}
                    for ffg in range(0, FFC, FFG):
                        for ffc in range(ffg, min(ffg + FFG, FFC)):
                            wr = p_wr.tile([P, OW], mm_dt)
                            nc.sync.dma_start(
                                wr, w_out[ffc * P:(ffc + 1) * P,
                                          oh * OW:(oh + 1) * OW])
                            for tb in range(TB):
                                nc.tensor.matmul(
                                    pss[tb], hT[:, ffc, tb * P:(tb + 1) * P],
                                    wr, start=(ffc == 0),
                                    stop=(ffc == FFC - 1))
                    for tb in range(TB):
                        nc.vector.tensor_add(
                            res3[:, tb, oh * OW:(oh + 1) * OW], pss[tb],
                            x2[:, tb, oh * OW:(oh + 1) * OW])

                xout = p_res.tile([P, TB, D], F32, tag="res")
                layernorm(res3, xout)
                outr = out.rearrange("(tb p) d -> p tb d", p=P)
                for tb in range(TB):
                    nc.sync.dma_start(outr[:, tb, :], xout[:, tb, :])

    nc.compile()
    return nc


# ---------------------------------------------------------------------------
# host side
# ---------------------------------------------------------------------------

_NC_CACHE = {}


def _get_nc(key=("f32",)):
    if key not in _NC_CACHE:
        if key == ("f32",):
            _NC_CACHE[key] = build_decoder_nc(mm_dt=F32)
        elif key == ("f32r",):
            _NC_CACHE[key] = build_decoder_nc(mm_dt=mybir.dt.float32r)
        else:
            raise KeyError(key)
    return _NC_CACHE[key]


MM_KEY = ("f32r",)  # f32r: full-rate PE (4x fp32) at ~1e-4 matmul rel err


def _numpy_reference(x, enc_out, src_mask, tgt_mask, wq1, bq1, wkv1, bkv1,
                     wo1, bo1, wq2, bq2, wkv2, bkv2, wo2, bo2, w_in, b_in,
                     w_out, b_out, g0, be0, g1, be1, g2, be2):
    """Pure-numpy fallback (exact reference semantics)."""
    H, D = 16, 1024

    def ln(x, g, b):
        m = x.mean(-1, keepdims=True)
        v = ((x - m) ** 2).mean(-1, keepdims=True)
        return (x - m) / np.sqrt(v + LN_EPS) * g + b

    def attn(q_in, mem, mask, wq, bq, wkv, bkv, wo, bo):
        B, T, _ = q_in.shape
        S = mem.shape[1]
        dhl = D // H
        q = (q_in @ wq + bq).reshape(B, T, H, dhl) * (dhl ** -0.5)
        k, v = np.split(mem @ wkv + bkv, 2, axis=-1)
        k = k.reshape(B, S, H, dhl)
        v = v.reshape(B, S, H, dhl)
        sc = np.einsum('bthd,bshd->bhts', q, k)
        sc = np.where(mask[:, None, :, :], -1e20, sc)
        sc = sc - sc.max(-1, keepdims=True)
        w = np.exp(sc)
        w = w / w.sum(-1, keepdims=True)
        ctx = np.einsum('bhts,bshd->bthd', w, v).reshape(B, T, D)
        return ctx @ wo + bo

    y = attn(x, x, tgt_mask, wq1, bq1, wkv1, bkv1, wo1, bo1)
    x1 = ln(x + y, g0, be0)
    y = attn(x1, enc_out, src_mask, wq2, bq2, wkv2, bkv2, wo2, bo2)
    x2 = ln(x1 + y, g1, be1)
    y = np.maximum(x2 @ w_in + b_in, 0.0) @ w_out + b_out
    return ln(x2 + y, g2, be2)


def kernel(x, enc_out, src_mask, tgt_mask, wq1, bq1, wkv1, bkv1, wo1, bo1,
           wq2, bq2, wkv2, bkv2, wo2, bo2, w_in, b_in, w_out, b_out,
           g0, be0, g1, be1, g2, be2, _trace=False):
    x = np.asarray(x)
    args = dict(x=x, enc_out=np.asarray(enc_out),
                src_mask=np.asarray(src_mask), tgt_mask=np.asarray(tgt_mask),
                wq1=np.asarray(wq1), bq1=np.asarray(bq1),
                wkv1=np.asarray(wkv1), bkv1=np.asarray(bkv1),
                wo1=np.asarray(wo1), bo1=np.asarray(bo1),
                wq2=np.asarray(wq2), bq2=np.asarray(bq2),
                wkv2=np.asarray(wkv2), bkv2=np.asarray(bkv2),
                wo2=np.asarray(wo2), bo2=np.asarray(bo2),
                w_in=np.asarray(w_in), b_in=np.asarray(b_in),
                w_out=np.asarray(w_out), b_out=np.asarray(b_out),
                g0=np.asarray(g0), be0=np.asarray(be0),
                g1=np.asarray(g1), be1=np.asarray(be1),
                g2=np.asarray(g2), be2=np.asarray(be2))

    # the hardware kernel folds out zero biases / unit gains (true for this
    # problem's setup_inputs); anything else falls back to exact numpy.
    zeros = [args[k] for k in ("bq1", "bkv1", "bo1", "bq2", "bkv2", "bo2",
                               "b_in", "b_out", "be0", "be1", "be2")]
    ones = [args["g0"], args["g1"], args["g2"]]
    if any(np.any(z != 0) for z in zeros) or any(np.any(g != 1) for g in ones):
        res = _numpy_reference(**args)
        return res.astype(np.float32), x

    B, T, D = x.shape
    TP = T // 2
    dh = D // 16
    sc = np.float32(dh ** -0.5)

    in_maps = []
    for core in range(8):
        b, half = divmod(core, 2)
        t0 = half * TP
        xb = args["x"][b]
        xs = xb[t0:t0 + TP]
        in_maps.append({
            "xqT": np.ascontiguousarray(xs.T),
            "xq": np.ascontiguousarray(xs),
            "xfT": np.ascontiguousarray(xb.T),
            "encT": np.ascontiguousarray(args["enc_out"][b].T),
            "m1T": np.ascontiguousarray(
                np.where(args["tgt_mask"][b, t0:t0 + TP], np.float32(-1e20),
                         np.float32(0)).T).astype(ml_dtypes.bfloat16),
            "m2T": np.ascontiguousarray(
                np.where(args["src_mask"][b, t0:t0 + TP], np.float32(-1e20),
                         np.float32(0)).T).astype(ml_dtypes.bfloat16),
            "wq1": args["wq1"] * sc,
            "wkv1": args["wkv1"],
            "wo1": args["wo1"],
            "wq2": args["wq2"] * sc,
            "wkv2": args["wkv2"],
            "wo2": args["wo2"],
            "w_in": args["w_in"],
            "w_out": args["w_out"],
        })

    nc = _get_nc(MM_KEY)
    res = run_bass_kernel_spmd(nc, in_maps, core_ids=list(range(8)),
                               trace=_trace)
    outp = np.empty((B, T, D), np.float32)
    for core in range(8):
        b, half = divmod(core, 2)
        outp[b, half * TP:(half + 1) * TP] = res.results[core]["out"]
    if _trace:
        kernel.last_results = res
    return outp, x
